# revision 10
# baseline (speedup 1.0000x reference)
"""Trainium2 Bass kernel for the ColorMemory block (v3).

Sharding: data-parallel over batch b across 8 NeuronCores (one batch element
per core); weights and the 512-row memory bank replicated per core.

All weight-only work is folded on the host:
  semP   = semantic_centers @ sem_w + sem_b                    [n, e]
  Wf     = (n1_w * q_w) @ semP^T                               [c, n]
  ncsum  = -sum_c Wf                                           [n]
  crow   = (n1_b @ q_w + q_b) @ semP^T                         [n]
  colemb = einsum('ind,bi->nd', host color-embed path, cls_b)  [n, ce]
  conv'  = n3_w * conv_w;  ccb = n3_b @ conv_w + conv_b

Device math per core (x stays feature-major [c, s]; LN1 is folded into the
logits as a rank-1 correction so the big matmul reads x straight from DRAM):
  l_raw[t,n] = x[:,t] @ Wf  +  mu1[t] * ncsum[n]  (+ sd1[t] * crow[n])
  p          = exp(rstd1 * l_raw - 96)            (no per-token max: softmax is
               shift-invariant and |logit| <= ~150 << the f32 exp range)
  cp         = (p^T)^T @ colemb / denom           [t, ce]
  y          = [x^T | cp], z2 = standardize(y)    (token-major, bf16)
  hT         = gelu(fc1^T @ z2T)                  (feature-major: no h transpose)
  mlp        = hT^T @ fc2, v = z2 + mlp
  outT       = conv'^T @ standardize(v)^T         [c, t] = native output layout

ACT table discipline: pass 1 uses only {Exp, Identity, Copy} (one table set),
pass 2 only {Gelu, Copy}; rstd everywhere is a DVE Newton iteration, so only
two table loads happen in the whole kernel.
"""

import numpy as np
import ml_dtypes
from contextlib import ExitStack

import concourse.bass as bass
import concourse.tile as tile
from concourse import bacc, mybir
from concourse.bass_utils import run_bass_kernel_spmd
from concourse.masks import make_identity

F32 = mybir.dt.float32
F32R = mybir.dt.float32r
BF16 = mybir.dt.bfloat16
I32 = mybir.dt.int32
AF = mybir.ActivationFunctionType
OP = mybir.AluOpType

N_CORES = 8
B, C, H, W = 8, 256, 64, 64
S = H * W              # 4096 tokens per core
NCOL = 512             # memory bank rows
CE = 256               # color embed dim
D2 = C + CE            # 512
EPS = 1e-5
P = 128

TW = 512               # token-tile width
NT = S // TW           # 8 tiles
NSUB = TW // P         # 4 subtiles per tile
CC = C // P            # 2 c-chunks
DC = D2 // P           # 4 chunks of the concat dim
NC_ = NCOL // P        # 4 n-chunks

LOGIT_SHIFT = -96.0    # replaces per-token max subtraction (see module doc)
RSQRT_MAGIC = 0x5F3759DF


def _newton_rstd(nc, pool, var_ap, ncols, eng=None, iters=2):
    """rstd [P, ncols] = (var+eps)^-0.5 via bit-magic + Newton steps."""
    if eng is None:
        eng = nc.vector
    a = pool.tile([P, ncols], F32, tag="nw_a")
    eng.tensor_scalar(out=a[:], in0=var_ap, scalar1=float(EPS),
                      scalar2=None, op0=OP.add)
    tb = pool.tile([P, ncols], I32, tag="nw_b")
    eng.tensor_scalar(out=tb[:], in0=a[:].bitcast(I32), scalar1=1,
                      scalar2=None, op0=OP.logical_shift_right)
    nb = pool.tile([P, ncols], I32, tag="nw_c")
    eng.tensor_scalar(out=nb[:], in0=tb[:], scalar1=RSQRT_MAGIC,
                      scalar2=-1, op0=OP.subtract, op1=OP.mult)
    y = nb[:].bitcast(F32)
    y2 = None
    for _ in range(iters):
        t = pool.tile([P, ncols], F32, tag="nw_t")
        eng.tensor_tensor(out=t[:], in0=y, in1=y, op=OP.mult)
        eng.tensor_tensor(out=t[:], in0=t[:], in1=a[:], op=OP.mult)
        eng.tensor_scalar(out=t[:], in0=t[:], scalar1=-0.5,
                          scalar2=1.5, op0=OP.mult, op1=OP.add)
        y2 = pool.tile([P, ncols], F32, tag="nw_y")
        eng.tensor_tensor(out=y2[:], in0=y, in1=t[:], op=OP.mult)
        y = y2[:]
    return y2


def build_bass(flags):
    nc = bacc.Bacc(
        "TRN2",
        target_bir_lowering=False,
        debug=False,
        enable_asserts=False,
        num_devices=N_CORES,
    )

    # ---- DRAM I/O (per-core shapes; weights pre-chunked on host) ----
    x_d = nc.dram_tensor("x", [C, S], F32R, kind="ExternalInput").ap()
    wf_d = nc.dram_tensor("wf", [P, CC, NCOL], F32R, kind="ExternalInput").ap()
    ncsum_d = nc.dram_tensor("ncsum", [1, NCOL], BF16, kind="ExternalInput").ap()
    colemb_d = nc.dram_tensor("colemb", [P, NC_, CE], BF16, kind="ExternalInput").ap()
    fc1_d = nc.dram_tensor("fc1", [P, DC, D2], BF16, kind="ExternalInput").ap()
    fc2_d = nc.dram_tensor("fc2", [P, DC, D2], BF16, kind="ExternalInput").ap()
    conv_d = nc.dram_tensor("conv", [P, DC, C], BF16, kind="ExternalInput").ap()
    opt = {}
    if flags["qcr"]:
        opt["crow"] = nc.dram_tensor("crow", [1, NCOL], BF16, kind="ExternalInput").ap()
    if flags["c1"]:
        opt["c1"] = nc.dram_tensor("c1b", [P, DC], F32, kind="ExternalInput").ap()
    if flags["fc2b"]:
        opt["fc2b"] = nc.dram_tensor("fc2b", [1, D2], BF16, kind="ExternalInput").ap()
    if flags["ln2w"]:
        opt["ln2w"] = nc.dram_tensor("ln2w", [P, D2], F32, kind="ExternalInput").ap()
    if flags["ln2b"]:
        opt["ln2b"] = nc.dram_tensor("ln2b", [P, D2], F32, kind="ExternalInput").ap()
    if flags["ccb"]:
        opt["ccb"] = nc.dram_tensor("ccb", [P, CC], F32, kind="ExternalInput").ap()
    out_d = nc.dram_tensor("out", [C, S], F32, kind="ExternalOutput").ap()

    with tile.TileContext(nc) as tc, ExitStack() as ctx:
        wpool = ctx.enter_context(tc.tile_pool(name="weights", bufs=1))
        z2pool = ctx.enter_context(tc.tile_pool(name="z2store", bufs=NT * NSUB))

        ident_f32 = wpool.tile([P, P], F32)
        make_identity(nc, ident_f32[:])
        ident_r = wpool.tile([P, P], F32R)
        nc.vector.tensor_copy(out=ident_r[:], in_=ident_f32[:])
        ident_b = wpool.tile([P, P], BF16)
        nc.vector.tensor_copy(out=ident_b[:], in_=ident_f32[:])

        shift_col = wpool.tile([P, 1], F32)
        nc.vector.memset(shift_col[:], LOGIT_SHIFT)

        wf_sb = wpool.tile([P, CC, NCOL], F32R)
        nc.sync.dma_start(out=wf_sb[:], in_=wf_d)
        ncsum_sb = wpool.tile([1, NCOL], BF16)
        nc.sync.dma_start(out=ncsum_sb[:], in_=ncsum_d)
        colemb_sb = wpool.tile([P, NC_, CE], BF16)
        nc.sync.dma_start(out=colemb_sb[:], in_=colemb_d)
        fc1_sb = wpool.tile([P, DC, D2], BF16)
        nc.sync.dma_start(out=fc1_sb[:], in_=fc1_d)
        fc2_sb = wpool.tile([P, DC, D2], BF16)
        nc.sync.dma_start(out=fc2_sb[:], in_=fc2_d)
        conv_sb = wpool.tile([P, DC, C], BF16)
        nc.sync.dma_start(out=conv_sb[:], in_=conv_d)

        bias_sb = {}
        for key, shape, dt in (("crow", [1, NCOL], BF16), ("c1", [P, DC], F32),
                               ("fc2b", [1, D2], BF16), ("ln2w", [P, D2], F32),
                               ("ln2b", [P, D2], F32), ("ccb", [P, CC], F32)):
            if key in opt:
                t = wpool.tile(shape, dt, name=f"b_{key}")
                nc.sync.dma_start(out=t[:], in_=opt[key])
                bias_sb[key] = t
        ones_bf = None
        if flags["fc2b"]:
            ones_bf = wpool.tile([1, P], BF16)
            nc.vector.memset(ones_bf[:], 1.0)

        z2_tiles = []

        # ================= pass 1: attention =================
        with (
            tc.tile_pool(name="p1x", bufs=3) as xpool,
            tc.tile_pool(name="p1y", bufs=6) as ypool,
            tc.tile_pool(name="p1p", bufs=3) as ppool,
            tc.tile_pool(name="p1pt", bufs=3) as ptpool,
            tc.tile_pool(name="p1row", bufs=4) as rowpool,
            tc.tile_pool(name="p1stats", bufs=8) as stats,
            tc.tile_pool(name="p1tpx", bufs=2, space="PSUM") as ps_xt,
            tc.tile_pool(name="p1row_ps", bufs=1, space="PSUM") as ps_row,
            tc.tile_pool(name="p1l", bufs=2, space="PSUM") as ps_l,
            tc.tile_pool(name="p1pt_ps", bufs=2, space="PSUM") as ps_pt,
            tc.tile_pool(name="p1cp", bufs=1, space="PSUM") as ps_cp,
        ):
            for T in range(NT):
                x_t = xpool.tile([P, CC, TW], F32R, tag="x")
                for cc in range(CC):
                    nc.sync.dma_start(
                        out=x_t[:, cc, :],
                        in_=x_d[cc * P:(cc + 1) * P, T * TW:(T + 1) * TW],
                    )
                # ---- stage A: x^T + LN1 stats for all 4 subtiles ----
                y_tile = []
                mv1 = stats.tile([P, NSUB, 2], F32, tag="mv1")
                for s in range(NSUB):
                    y_t = ypool.tile([P, D2], BF16, tag="y")
                    tpx = ps_xt.tile([P, CC, P], F32R, tag="tpx")
                    for cc in range(CC):
                        nc.tensor.transpose(
                            out=tpx[:, cc, :],
                            in_=x_t[:, cc, s * P:(s + 1) * P],
                            identity=ident_r[:],
                        )
                    nc.any.tensor_copy(out=y_t[:, 0:C], in_=tpx[:])
                    st1 = stats.tile([P, nc.vector.BN_STATS_DIM], F32, tag="st1")
                    nc.vector.bn_stats(out=st1[:], in_=y_t[:, 0:C])
                    nc.vector.bn_aggr(out=mv1[:, s, :], in_=st1[:])
                    y_tile.append(y_t)
                rstd1 = _newton_rstd(nc, stats, mv1[:, :, 1], NSUB)
                # ---- stage B: logits + softmax + color prior ----
                mv2 = stats.tile([P, NSUB, 2], F32, tag="mv2")
                for s in range(NSUB):
                    y_t = y_tile[s]
                    mub = stats.tile([P, 1], BF16, tag="mub")
                    nc.gpsimd.tensor_copy(out=mub[:], in_=mv1[:, s, 0:1])
                    rowp = ps_row.tile([1, P], F32, tag="rowp")
                    nc.tensor.matmul(out=rowp[:], lhsT=mub[:], rhs=ident_b[:],
                                     start=True, stop=True)
                    murow = rowpool.tile([1, P], BF16, tag="murow")
                    nc.scalar.copy(out=murow[:], in_=rowp[:])
                    if flags["qcr"]:
                        sd = stats.tile([P, 1], F32, tag="sd")
                        nc.vector.reciprocal(out=sd[:], in_=rstd1[:, s:s + 1])
                        sdb = stats.tile([P, 1], BF16, tag="sdb")
                        nc.vector.tensor_copy(out=sdb[:], in_=sd[:])
                        rowp2 = ps_row.tile([1, P], F32, tag="rowp2")
                        nc.tensor.matmul(out=rowp2[:], lhsT=sdb[:],
                                         rhs=ident_b[:], start=True, stop=True)
                        sdrow = rowpool.tile([1, P], BF16, tag="sdrow")
                        nc.scalar.copy(out=sdrow[:], in_=rowp2[:])
                    psl = ps_l.tile([P, NCOL], F32, tag="l")
                    nc.tensor.matmul(out=psl[:],
                                     lhsT=x_t[:, 0, s * P:(s + 1) * P],
                                     rhs=wf_sb[:, 0, :], start=True, stop=False)
                    nc.tensor.matmul(out=psl[:],
                                     lhsT=x_t[:, 1, s * P:(s + 1) * P],
                                     rhs=wf_sb[:, 1, :], start=False, stop=False)
                    nc.tensor.matmul(out=psl[:], lhsT=murow[:], rhs=ncsum_sb[:],
                                     start=False, stop=not flags["qcr"])
                    if flags["qcr"]:
                        nc.tensor.matmul(out=psl[:], lhsT=sdrow[:],
                                         rhs=bias_sb["crow"][:],
                                         start=False, stop=True)
                    p_sb = ppool.tile([P, NCOL], BF16, tag="p")
                    denom = stats.tile([P, 1], F32, tag="denom")
                    nc.scalar.activation(out=p_sb[:], in_=psl[:], func=AF.Exp,
                                         bias=shift_col[:],
                                         scale=rstd1[:, s:s + 1],
                                         accum_out=denom[:])
                    recip = stats.tile([P, 1], F32, tag="recip")
                    nc.vector.reciprocal(out=recip[:], in_=denom[:])
                    ptp = ps_pt.tile([P, NC_, P], BF16, tag="ptp")
                    for j in range(NC_):
                        nc.tensor.transpose(out=ptp[:, j, :],
                                            in_=p_sb[:, j * P:(j + 1) * P],
                                            identity=ident_b[:])
                    pt_sb = ptpool.tile([P, NC_, P], BF16, tag="pt")
                    nc.scalar.copy(out=pt_sb[:], in_=ptp[:])
                    pcp = ps_cp.tile([P, CE], F32, tag="cp")
                    for j in range(NC_):
                        nc.tensor.matmul(out=pcp[:], lhsT=pt_sb[:, j, :],
                                         rhs=colemb_sb[:, j, :],
                                         start=(j == 0), stop=(j == NC_ - 1))
                    nc.scalar.activation(out=y_t[:, C:D2], in_=pcp[:],
                                         func=AF.Identity, scale=recip[:])
                    st2 = stats.tile([P, nc.vector.BN_STATS_DIM], F32, tag="st2")
                    nc.vector.bn_stats(out=st2[:], in_=y_t[:])
                    nc.vector.bn_aggr(out=mv2[:, s, :], in_=st2[:])
                # ---- stage C: LN2 -> z2 (gpsimd) ----
                rstd2 = _newton_rstd(nc, stats, mv2[:, :, 1], NSUB)
                for s in range(NSUB):
                    z2_t = z2pool.tile([P, D2], BF16, tag="z2")
                    nc.gpsimd.tensor_scalar(out=z2_t[:], in0=y_tile[s][:],
                                            scalar1=mv2[:, s, 0:1],
                                            scalar2=rstd2[:, s:s + 1],
                                            op0=OP.subtract, op1=OP.mult)
                    if flags["ln2w"]:
                        nc.gpsimd.tensor_tensor(out=z2_t[:], in0=z2_t[:],
                                                in1=bias_sb["ln2w"][:],
                                                op=OP.mult)
                    if flags["ln2b"]:
                        nc.gpsimd.tensor_tensor(out=z2_t[:], in0=z2_t[:],
                                                in1=bias_sb["ln2b"][:],
                                                op=OP.add)
                    z2_tiles.append(z2_t)

        tc.no_sync_barrier()

        # ================= pass 2: MLP (gelu) =================
        with (
            tc.tile_pool(name="p2z2T", bufs=2) as z2Tpool,
            tc.tile_pool(name="p2hT", bufs=2) as hTpool,
            tc.tile_pool(name="p2tz", bufs=2, space="PSUM") as ps_tz,
            tc.tile_pool(name="p2h", bufs=3, space="PSUM") as ps_h,
            tc.tile_pool(name="p2m", bufs=2, space="PSUM") as ps_m,
        ):
            for T in range(NT):
                z2T = z2Tpool.tile([P, DC, TW], BF16, tag="z2T")
                for f in range(DC):
                    tz = ps_tz.tile([P, TW], BF16, tag="tz")
                    for s in range(NSUB):
                        nc.tensor.transpose(
                            out=tz[:, s * P:(s + 1) * P],
                            in_=z2_tiles[T * NSUB + s][:, f * P:(f + 1) * P],
                            identity=ident_b[:])
                    nc.any.tensor_copy(out=z2T[:, f, :], in_=tz[:])
                # hT = gelu(fc1^T @ z2T): feature-major, no transpose of h
                hT = hTpool.tile([P, DC, TW], BF16, tag="hT")
                for o in range(DC):
                    ph = ps_h.tile([P, TW], F32, tag="h")
                    for f in range(DC):
                        nc.tensor.matmul(out=ph[:],
                                         lhsT=fc1_sb[:, f, o * P:(o + 1) * P],
                                         rhs=z2T[:, f, :],
                                         start=(f == 0), stop=(f == DC - 1))
                    bias = bias_sb["c1"][:, o:o + 1] if flags["c1"] else 0.0
                    nc.scalar.activation(out=hT[:, o, :], in_=ph[:], func=AF.Gelu,
                                         bias=bias)
                # mlp (token-major) + residual, v overwrites the z2 slot
                for s in range(NSUB):
                    pm = ps_m.tile([P, TW], F32, tag="m")
                    for o in range(DC):
                        nc.tensor.matmul(out=pm[:],
                                         lhsT=hT[:, o, s * P:(s + 1) * P],
                                         rhs=fc2_sb[:, o, :], start=(o == 0),
                                         stop=(o == DC - 1) and not flags["fc2b"])
                    if flags["fc2b"]:
                        nc.tensor.matmul(out=pm[:], lhsT=ones_bf[:],
                                         rhs=bias_sb["fc2b"][:],
                                         start=False, stop=True)
                    z2_t = z2_tiles[T * NSUB + s]
                    nc.vector.tensor_tensor(out=z2_t[:], in0=z2_t[:], in1=pm[:],
                                            op=OP.add)

        tc.no_sync_barrier()

        # ================= pass 3: LN3 + output conv =================
        with (
            tc.tile_pool(name="p3z3", bufs=6) as z3pool,
            tc.tile_pool(name="p3z3T", bufs=2) as z3Tpool,
            tc.tile_pool(name="p3o", bufs=4) as opool,
            tc.tile_pool(name="p3stats", bufs=4) as stats3,
            tc.tile_pool(name="p3tz", bufs=2, space="PSUM") as ps_tz3,
            tc.tile_pool(name="p3o_ps", bufs=2, space="PSUM") as ps_o,
        ):
            for T in range(NT):
                mv3 = stats3.tile([P, NSUB, 2], F32, tag="mv3")
                for s in range(NSUB):
                    st3 = stats3.tile([P, nc.vector.BN_STATS_DIM], F32, tag="st3")
                    nc.vector.bn_stats(out=st3[:], in_=z2_tiles[T * NSUB + s][:])
                    nc.vector.bn_aggr(out=mv3[:, s, :], in_=st3[:])
                rstd3 = _newton_rstd(nc, stats3, mv3[:, :, 1], NSUB)
                z3_list = []
                for s in range(NSUB):
                    z3_t = z3pool.tile([P, D2], BF16, tag="z3")
                    nc.gpsimd.tensor_scalar(out=z3_t[:],
                                            in0=z2_tiles[T * NSUB + s][:],
                                            scalar1=mv3[:, s, 0:1],
                                            scalar2=rstd3[:, s:s + 1],
                                            op0=OP.subtract, op1=OP.mult)
                    z3_list.append(z3_t)
                z3T = z3Tpool.tile([P, DC, TW], BF16, tag="z3T")
                for f in range(DC):
                    tz = ps_tz3.tile([P, TW], BF16, tag="tz3")
                    for s in range(NSUB):
                        nc.tensor.transpose(
                            out=tz[:, s * P:(s + 1) * P],
                            in_=z3_list[s][:, f * P:(f + 1) * P],
                            identity=ident_b[:])
                    nc.any.tensor_copy(out=z3T[:, f, :], in_=tz[:])
                for cc in range(CC):
                    po = ps_o.tile([P, TW], F32, tag="o")
                    for f in range(DC):
                        nc.tensor.matmul(out=po[:],
                                         lhsT=conv_sb[:, f, cc * P:(cc + 1) * P],
                                         rhs=z3T[:, f, :],
                                         start=(f == 0), stop=(f == DC - 1))
                    ot = opool.tile([P, TW], F32, tag="ot")
                    if flags["ccb"]:
                        nc.any.tensor_scalar(out=ot[:], in0=po[:],
                                             scalar1=bias_sb["ccb"][:, cc:cc + 1],
                                             scalar2=None, op0=OP.add)
                    else:
                        nc.any.tensor_copy(out=ot[:], in_=po[:])
                    nc.sync.dma_start(
                        out=out_d[cc * P:(cc + 1) * P, T * TW:(T + 1) * TW],
                        in_=ot[:])

    nc.compile()
    return nc


_CACHE = {}


def _chunk(a, p=P):
    """[K, N] -> [P, K//P, N] (k-chunks on partitions)."""
    k, n = a.shape
    return np.ascontiguousarray(a.reshape(k // p, p, n).transpose(1, 0, 2))


def _prep_inputs_impl(x, cls, color_centers, semantic_centers, a_embed, b_embed,
                      ce_w, ce_b, sem_w, sem_b, q_w, q_b,
                      n1_w, n1_b, n2_w, n2_b, n3_w, n3_b,
                      fc1_w, fc1_b, fc2_w, fc2_b, conv_w, conv_b):
    f32 = lambda a: np.asarray(a, np.float32)
    bf = lambda a: np.ascontiguousarray(np.asarray(a, ml_dtypes.bfloat16))
    x = np.ascontiguousarray(f32(x))
    cls = f32(cls)
    color_centers = np.asarray(color_centers, np.int64)
    semantic_centers = f32(semantic_centers)
    a_embed, b_embed = f32(a_embed), f32(b_embed)
    ce_w, ce_b = f32(ce_w), f32(ce_b)
    sem_w, sem_b = f32(sem_w), f32(sem_b)
    q_w, q_b = f32(q_w), f32(q_b)
    n1_w, n1_b = f32(n1_w), f32(n1_b)
    n2_w, n2_b = f32(n2_w), f32(n2_b)
    n3_w, n3_b = f32(n3_w), f32(n3_b)
    fc1_w, fc1_b = f32(fc1_w), f32(fc1_b)
    fc2_w, fc2_b = f32(fc2_w), f32(fc2_b)
    conv_w, conv_b = f32(conv_w), f32(conv_b)

    # ---- host-side weight folding ----
    semP = semantic_centers @ sem_w + sem_b                  # [n, e]
    Wf = (n1_w[:, None] * q_w) @ semP.T                      # [c, n]
    ncsum = -Wf.sum(0)                                       # [n]
    crow = (n1_b @ q_w + q_b) @ semP.T                       # [n]
    ab = np.concatenate([a_embed[color_centers[:, :, 0]],
                         b_embed[color_centers[:, :, 1]]], -1)   # [4, n, 2ce]
    ce = np.einsum('inf,ifd->ind', ab, ce_w) + ce_b[:, None, :]  # [4, n, ce]
    colemb_all = np.einsum('ind,bi->bnd', ce, cls)               # [b, n, ce]
    conv_f = n3_w[:, None] * conv_w
    ccb = n3_b @ conv_w + conv_b

    nz = lambda a: bool(np.any(a != 0))
    flags = {
        "qcr": nz(crow),
        "c1": nz(fc1_b),
        "fc2b": nz(fc2_b),
        "ln2w": bool(np.any(n2_w != 1.0)),
        "ln2b": nz(n2_b),
        "ccb": nz(ccb),
    }

    wf_p = _chunk(Wf)                                        # [P, CC, NCOL] f32
    fc1_p = bf(_chunk(fc1_w))                                # [P, DC, D2]
    fc2_p = bf(_chunk(fc2_w))
    conv_p = bf(_chunk(conv_f))                              # [P, DC, C]
    ncsum_p = bf(ncsum[None, :])

    xn = x.reshape(B, C, S)
    in_maps = []
    for k in range(N_CORES):
        m = {
            "x": np.ascontiguousarray(xn[k]),
            "wf": wf_p,
            "ncsum": ncsum_p,
            "colemb": bf(_chunk(colemb_all[k])),             # [P, NC_, CE]
            "fc1": fc1_p,
            "fc2": fc2_p,
            "conv": conv_p,
        }
        if flags["qcr"]:
            m["crow"] = bf(crow[None, :])
        if flags["c1"]:
            m["c1b"] = np.ascontiguousarray(fc1_b.reshape(DC, P).T)
        if flags["fc2b"]:
            m["fc2b"] = bf(fc2_b[None, :])
        if flags["ln2w"]:
            m["ln2w"] = np.ascontiguousarray(np.broadcast_to(n2_w, (P, D2)))
        if flags["ln2b"]:
            m["ln2b"] = np.ascontiguousarray(np.broadcast_to(n2_b, (P, D2)))
        if flags["ccb"]:
            m["ccb"] = np.ascontiguousarray(ccb.reshape(CC, P).T)
        in_maps.append(m)
    return flags, in_maps


def run(flags, in_maps, **kw):
    key = tuple(sorted(flags.items()))
    if key not in _CACHE:
        _CACHE[key] = build_bass(flags)
    nc = _CACHE[key]
    res = run_bass_kernel_spmd(nc, in_maps, core_ids=list(range(N_CORES)), **kw)
    out = np.stack([res.results[k]["out"] for k in range(N_CORES)], axis=0)
    return out.reshape(B, C, H, W), res


def kernel(**inputs):
    flags, in_maps = _prep_inputs(**inputs)
    out, _ = run(flags, in_maps)
    return out


def _prep_inputs(x, cls, color_centers, semantic_centers, a_embed, b_embed,
                 ce_w, ce_b, sem_w, sem_b, q_w, q_b,
                 n1_w, n1_b, n2_w, n2_b, n3_w, n3_b,
                 fc1_w, fc1_b, fc2_w, fc2_b, conv_w, conv_b):
    return _prep_inputs_impl(
        x, cls, color_centers, semantic_centers, a_embed, b_embed,
        ce_w, ce_b, sem_w, sem_b, q_w, q_b,
        n1_w, n1_b, n2_w, n2_b, n3_w, n3_b,
        fc1_w, fc1_b, fc2_w, fc2_b, conv_w, conv_b)


# revision 11
# speedup vs baseline: 2.2966x; 2.2966x over previous
"""Trainium2 Bass kernel for the ColorMemory block (v3).

Sharding: data-parallel over batch b across 8 NeuronCores (one batch element
per core); weights and the 512-row memory bank replicated per core.

All weight-only work is folded on the host:
  semP   = semantic_centers @ sem_w + sem_b                    [n, e]
  Wf     = (n1_w * q_w) @ semP^T                               [c, n]
  ncsum  = -sum_c Wf                                           [n]
  crow   = (n1_b @ q_w + q_b) @ semP^T                         [n]
  colemb = einsum('ind,bi->nd', host color-embed path, cls_b)  [n, ce]
  conv'  = n3_w * conv_w;  ccb = n3_b @ conv_w + conv_b

Device math per core (x stays feature-major [c, s]; LN1 is folded into the
logits as a rank-1 correction so the big matmul reads x straight from DRAM):
  l_raw[t,n] = x[:,t] @ Wf  +  mu1[t] * ncsum[n]  (+ sd1[t] * crow[n])
  p          = exp(rstd1 * l_raw - 96)            (no per-token max: softmax is
               shift-invariant and |logit| <= ~150 << the f32 exp range)
  cp         = (p^T)^T @ colemb / denom           [t, ce]
  y          = [x^T | cp], z2 = standardize(y)    (token-major, bf16)
  hT         = gelu(fc1^T @ z2T)                  (feature-major: no h transpose)
  mlp        = hT^T @ fc2, v = z2 + mlp
  outT       = conv'^T @ standardize(v)^T         [c, t] = native output layout

ACT table discipline: pass 1 uses only {Exp, Identity, Copy} (one table set),
pass 2 only {Gelu, Copy}; rstd everywhere is a DVE Newton iteration, so only
two table loads happen in the whole kernel.
"""

import numpy as np
import ml_dtypes
from contextlib import ExitStack

import concourse.bass as bass
import concourse.tile as tile
from concourse import bacc, mybir
from concourse.bass_utils import run_bass_kernel_spmd
from concourse.masks import make_identity

F32 = mybir.dt.float32
F32R = mybir.dt.float32r
BF16 = mybir.dt.bfloat16
I32 = mybir.dt.int32
AF = mybir.ActivationFunctionType
OP = mybir.AluOpType

N_CORES = 8
B, C, H, W = 8, 256, 64, 64
S = H * W              # 4096 tokens per core
NCOL = 512             # memory bank rows
CE = 256               # color embed dim
D2 = C + CE            # 512
EPS = 1e-5
P = 128

TW = 512               # token-tile width
NT = S // TW           # 8 tiles
NSUB = TW // P         # 4 subtiles per tile
CC = C // P            # 2 c-chunks
DC = D2 // P           # 4 chunks of the concat dim
NC_ = NCOL // P        # 4 n-chunks

LOGIT_SHIFT = -96.0    # replaces per-token max subtraction (see module doc)
RSQRT_MAGIC = 0x5F3759DF


def _newton_rstd(nc, pool, var_ap, ncols, eng=None, iters=2):
    """rstd [P, ncols] = (var+eps)^-0.5 via bit-magic + Newton steps."""
    if eng is None:
        eng = nc.vector
    a = pool.tile([P, ncols], F32, tag="nw_a")
    eng.tensor_scalar(out=a[:], in0=var_ap, scalar1=float(EPS),
                      scalar2=None, op0=OP.add)
    tb = pool.tile([P, ncols], I32, tag="nw_b")
    eng.tensor_scalar(out=tb[:], in0=a[:].bitcast(I32), scalar1=1,
                      scalar2=None, op0=OP.logical_shift_right)
    nb = pool.tile([P, ncols], I32, tag="nw_c")
    eng.tensor_scalar(out=nb[:], in0=tb[:], scalar1=RSQRT_MAGIC,
                      scalar2=-1, op0=OP.subtract, op1=OP.mult)
    y = nb[:].bitcast(F32)
    y2 = None
    for _ in range(iters):
        t = pool.tile([P, ncols], F32, tag="nw_t")
        eng.tensor_tensor(out=t[:], in0=y, in1=y, op=OP.mult)
        eng.tensor_tensor(out=t[:], in0=t[:], in1=a[:], op=OP.mult)
        eng.tensor_scalar(out=t[:], in0=t[:], scalar1=-0.5,
                          scalar2=1.5, op0=OP.mult, op1=OP.add)
        y2 = pool.tile([P, ncols], F32, tag="nw_y")
        eng.tensor_tensor(out=y2[:], in0=y, in1=t[:], op=OP.mult)
        y = y2[:]
    return y2


def build_bass(flags):
    nc = bacc.Bacc(
        "TRN2",
        target_bir_lowering=False,
        debug=False,
        enable_asserts=False,
        num_devices=N_CORES,
    )

    # ---- DRAM I/O (per-core shapes; weights pre-chunked on host) ----
    x_d = nc.dram_tensor("x", [C, S], F32R, kind="ExternalInput").ap()
    wf_d = nc.dram_tensor("wf", [P, CC, NCOL], F32R, kind="ExternalInput").ap()
    ncsum_d = nc.dram_tensor("ncsum", [1, NCOL], BF16, kind="ExternalInput").ap()
    colemb_d = nc.dram_tensor("colemb", [P, NC_, CE], BF16, kind="ExternalInput").ap()
    fc1_d = nc.dram_tensor("fc1", [P, DC, D2], BF16, kind="ExternalInput").ap()
    fc2_d = nc.dram_tensor("fc2", [P, DC, D2], BF16, kind="ExternalInput").ap()
    conv_d = nc.dram_tensor("conv", [P, DC, C], BF16, kind="ExternalInput").ap()
    opt = {}
    if flags["qcr"]:
        opt["crow"] = nc.dram_tensor("crow", [1, NCOL], BF16, kind="ExternalInput").ap()
    if flags["c1"]:
        opt["c1"] = nc.dram_tensor("c1b", [P, DC], F32, kind="ExternalInput").ap()
    if flags["fc2b"]:
        opt["fc2b"] = nc.dram_tensor("fc2b", [1, D2], BF16, kind="ExternalInput").ap()
    if flags["ln2w"]:
        opt["ln2w"] = nc.dram_tensor("ln2w", [P, D2], F32, kind="ExternalInput").ap()
    if flags["ln2b"]:
        opt["ln2b"] = nc.dram_tensor("ln2b", [P, D2], F32, kind="ExternalInput").ap()
    if flags["ccb"]:
        opt["ccb"] = nc.dram_tensor("ccb", [P, CC], F32, kind="ExternalInput").ap()
    out_d = nc.dram_tensor("out", [C, S], F32, kind="ExternalOutput").ap()

    with tile.TileContext(nc) as tc, ExitStack() as ctx:
        wpool = ctx.enter_context(tc.tile_pool(name="weights", bufs=1))
        z2pool = ctx.enter_context(tc.tile_pool(name="z2store", bufs=NT * NSUB))

        ident_f32 = wpool.tile([P, P], F32)
        make_identity(nc, ident_f32[:])
        ident_r = wpool.tile([P, P], F32R)
        nc.vector.tensor_copy(out=ident_r[:], in_=ident_f32[:])
        ident_b = wpool.tile([P, P], BF16)
        nc.vector.tensor_copy(out=ident_b[:], in_=ident_f32[:])

        shift_col = wpool.tile([P, 1], F32)
        nc.vector.memset(shift_col[:], LOGIT_SHIFT)

        wf_sb = wpool.tile([P, CC, NCOL], F32R)
        nc.sync.dma_start(out=wf_sb[:], in_=wf_d)
        ncsum_sb = wpool.tile([1, NCOL], BF16)
        nc.sync.dma_start(out=ncsum_sb[:], in_=ncsum_d)
        colemb_sb = wpool.tile([P, NC_, CE], BF16)
        nc.sync.dma_start(out=colemb_sb[:], in_=colemb_d)
        fc1_sb = wpool.tile([P, DC, D2], BF16)
        nc.sync.dma_start(out=fc1_sb[:], in_=fc1_d)
        fc2_sb = wpool.tile([P, DC, D2], BF16)
        nc.sync.dma_start(out=fc2_sb[:], in_=fc2_d)
        conv_sb = wpool.tile([P, DC, C], BF16)
        nc.sync.dma_start(out=conv_sb[:], in_=conv_d)

        bias_sb = {}
        for key, shape, dt in (("crow", [1, NCOL], BF16), ("c1", [P, DC], F32),
                               ("fc2b", [1, D2], BF16), ("ln2w", [P, D2], F32),
                               ("ln2b", [P, D2], F32), ("ccb", [P, CC], F32)):
            if key in opt:
                t = wpool.tile(shape, dt, name=f"b_{key}")
                nc.sync.dma_start(out=t[:], in_=opt[key])
                bias_sb[key] = t
        ones_bf = None
        if flags["fc2b"]:
            ones_bf = wpool.tile([1, P], BF16)
            nc.vector.memset(ones_bf[:], 1.0)

        z2_tiles = []

        # ================= pass 1: attention =================
        with (
            tc.tile_pool(name="p1x", bufs=3) as xpool,
            tc.tile_pool(name="p1y", bufs=6) as ypool,
            tc.tile_pool(name="p1p", bufs=3) as ppool,
            tc.tile_pool(name="p1pt", bufs=3) as ptpool,
            tc.tile_pool(name="p1row", bufs=4) as rowpool,
            tc.tile_pool(name="p1stats", bufs=8) as stats,
            tc.tile_pool(name="p1tpx", bufs=2, space="PSUM") as ps_xt,
            tc.tile_pool(name="p1row_ps", bufs=1, space="PSUM") as ps_row,
            tc.tile_pool(name="p1l", bufs=2, space="PSUM") as ps_l,
            tc.tile_pool(name="p1pt_ps", bufs=2, space="PSUM") as ps_pt,
            tc.tile_pool(name="p1cp", bufs=1, space="PSUM") as ps_cp,
        ):
            for T in range(NT):
                x_t = xpool.tile([P, CC, TW], F32R, tag="x")
                for cc in range(CC):
                    nc.sync.dma_start(
                        out=x_t[:, cc, :],
                        in_=x_d[cc * P:(cc + 1) * P, T * TW:(T + 1) * TW],
                    )
                # ---- stage A: x^T + LN1 stats for all 4 subtiles ----
                y_tile = []
                mv1 = stats.tile([P, NSUB, 2], F32, tag="mv1")
                for s in range(NSUB):
                    y_t = ypool.tile([P, D2], BF16, tag="y")
                    tpx = ps_xt.tile([P, CC, P], F32R, tag="tpx")
                    for cc in range(CC):
                        nc.tensor.transpose(
                            out=tpx[:, cc, :],
                            in_=x_t[:, cc, s * P:(s + 1) * P],
                            identity=ident_r[:],
                        )
                    nc.any.tensor_copy(out=y_t[:, 0:C], in_=tpx[:])
                    st1 = stats.tile([P, nc.vector.BN_STATS_DIM], F32, tag="st1")
                    nc.vector.bn_stats(out=st1[:], in_=y_t[:, 0:C])
                    nc.vector.bn_aggr(out=mv1[:, s, :], in_=st1[:])
                    y_tile.append(y_t)
                rstd1 = _newton_rstd(nc, stats, mv1[:, :, 1], NSUB)
                # ---- stage B: logits + softmax + color prior ----
                mv2 = stats.tile([P, NSUB, 2], F32, tag="mv2")
                for s in range(NSUB):
                    y_t = y_tile[s]
                    mub = stats.tile([P, 1], BF16, tag="mub")
                    nc.vector.tensor_copy(out=mub[:], in_=mv1[:, s, 0:1])
                    rowp = ps_row.tile([1, P], F32, tag="rowp")
                    nc.tensor.matmul(out=rowp[:], lhsT=mub[:], rhs=ident_b[:],
                                     start=True, stop=True)
                    murow = rowpool.tile([1, P], BF16, tag="murow")
                    nc.scalar.copy(out=murow[:], in_=rowp[:])
                    if flags["qcr"]:
                        sd = stats.tile([P, 1], F32, tag="sd")
                        nc.vector.reciprocal(out=sd[:], in_=rstd1[:, s:s + 1])
                        sdb = stats.tile([P, 1], BF16, tag="sdb")
                        nc.vector.tensor_copy(out=sdb[:], in_=sd[:])
                        rowp2 = ps_row.tile([1, P], F32, tag="rowp2")
                        nc.tensor.matmul(out=rowp2[:], lhsT=sdb[:],
                                         rhs=ident_b[:], start=True, stop=True)
                        sdrow = rowpool.tile([1, P], BF16, tag="sdrow")
                        nc.scalar.copy(out=sdrow[:], in_=rowp2[:])
                    psl = ps_l.tile([P, NCOL], F32, tag="l")
                    nc.tensor.matmul(out=psl[:],
                                     lhsT=x_t[:, 0, s * P:(s + 1) * P],
                                     rhs=wf_sb[:, 0, :], start=True, stop=False)
                    nc.tensor.matmul(out=psl[:],
                                     lhsT=x_t[:, 1, s * P:(s + 1) * P],
                                     rhs=wf_sb[:, 1, :], start=False, stop=False)
                    nc.tensor.matmul(out=psl[:], lhsT=murow[:], rhs=ncsum_sb[:],
                                     start=False, stop=not flags["qcr"])
                    if flags["qcr"]:
                        nc.tensor.matmul(out=psl[:], lhsT=sdrow[:],
                                         rhs=bias_sb["crow"][:],
                                         start=False, stop=True)
                    p_sb = ppool.tile([P, NCOL], BF16, tag="p")
                    denom = stats.tile([P, 1], F32, tag="denom")
                    nc.scalar.activation(out=p_sb[:], in_=psl[:], func=AF.Exp,
                                         bias=shift_col[:],
                                         scale=rstd1[:, s:s + 1],
                                         accum_out=denom[:])
                    recip = stats.tile([P, 1], F32, tag="recip")
                    nc.vector.reciprocal(out=recip[:], in_=denom[:])
                    ptp = ps_pt.tile([P, NC_, P], BF16, tag="ptp")
                    for j in range(NC_):
                        nc.tensor.transpose(out=ptp[:, j, :],
                                            in_=p_sb[:, j * P:(j + 1) * P],
                                            identity=ident_b[:])
                    pt_sb = ptpool.tile([P, NC_, P], BF16, tag="pt")
                    nc.scalar.copy(out=pt_sb[:], in_=ptp[:])
                    pcp = ps_cp.tile([P, CE], F32, tag="cp")
                    for j in range(NC_):
                        nc.tensor.matmul(out=pcp[:], lhsT=pt_sb[:, j, :],
                                         rhs=colemb_sb[:, j, :],
                                         start=(j == 0), stop=(j == NC_ - 1))
                    nc.scalar.activation(out=y_t[:, C:D2], in_=pcp[:],
                                         func=AF.Identity, scale=recip[:])
                    st2 = stats.tile([P, nc.vector.BN_STATS_DIM], F32, tag="st2")
                    nc.vector.bn_stats(out=st2[:], in_=y_t[:])
                    nc.vector.bn_aggr(out=mv2[:, s, :], in_=st2[:])
                # ---- stage C: LN2 -> z2 (gpsimd) ----
                rstd2 = _newton_rstd(nc, stats, mv2[:, :, 1], NSUB)
                nmr2 = stats.tile([P, NSUB], F32, tag="nmr2")
                nc.vector.tensor_tensor(out=nmr2[:], in0=mv2[:, :, 0],
                                        in1=rstd2[:], op=OP.mult)
                nc.vector.tensor_scalar(out=nmr2[:], in0=nmr2[:], scalar1=-1.0,
                                        scalar2=None, op0=OP.mult)
                for s in range(NSUB):
                    z2_t = z2pool.tile([P, D2], BF16, tag="z2")
                    nc.scalar.activation(out=z2_t[:], in_=y_tile[s][:],
                                         func=AF.Identity,
                                         scale=rstd2[:, s:s + 1],
                                         bias=nmr2[:, s:s + 1])
                    if flags["ln2w"]:
                        nc.vector.tensor_tensor(out=z2_t[:], in0=z2_t[:],
                                                in1=bias_sb["ln2w"][:],
                                                op=OP.mult)
                    if flags["ln2b"]:
                        nc.vector.tensor_tensor(out=z2_t[:], in0=z2_t[:],
                                                in1=bias_sb["ln2b"][:],
                                                op=OP.add)
                    z2_tiles.append(z2_t)

        tc.no_sync_barrier()

        # ================= pass 2: MLP (gelu) =================
        with (
            tc.tile_pool(name="p2z2T", bufs=2) as z2Tpool,
            tc.tile_pool(name="p2hT", bufs=2) as hTpool,
            tc.tile_pool(name="p2tz", bufs=2, space="PSUM") as ps_tz,
            tc.tile_pool(name="p2h", bufs=3, space="PSUM") as ps_h,
            tc.tile_pool(name="p2m", bufs=2, space="PSUM") as ps_m,
        ):
            for T in range(NT):
                z2T = z2Tpool.tile([P, DC, TW], BF16, tag="z2T")
                for f in range(DC):
                    tz = ps_tz.tile([P, TW], BF16, tag="tz")
                    for s in range(NSUB):
                        nc.tensor.transpose(
                            out=tz[:, s * P:(s + 1) * P],
                            in_=z2_tiles[T * NSUB + s][:, f * P:(f + 1) * P],
                            identity=ident_b[:])
                    nc.any.tensor_copy(out=z2T[:, f, :], in_=tz[:])
                # hT = gelu(fc1^T @ z2T): feature-major, no transpose of h
                hT = hTpool.tile([P, DC, TW], BF16, tag="hT")
                for o in range(DC):
                    ph = ps_h.tile([P, TW], F32, tag="h")
                    for f in range(DC):
                        nc.tensor.matmul(out=ph[:],
                                         lhsT=fc1_sb[:, f, o * P:(o + 1) * P],
                                         rhs=z2T[:, f, :],
                                         start=(f == 0), stop=(f == DC - 1))
                    bias = bias_sb["c1"][:, o:o + 1] if flags["c1"] else 0.0
                    nc.scalar.activation(out=hT[:, o, :], in_=ph[:], func=AF.Gelu,
                                         bias=bias)
                # mlp (token-major) + residual, v overwrites the z2 slot
                for s in range(NSUB):
                    pm = ps_m.tile([P, TW], F32, tag="m")
                    for o in range(DC):
                        nc.tensor.matmul(out=pm[:],
                                         lhsT=hT[:, o, s * P:(s + 1) * P],
                                         rhs=fc2_sb[:, o, :], start=(o == 0),
                                         stop=(o == DC - 1) and not flags["fc2b"])
                    if flags["fc2b"]:
                        nc.tensor.matmul(out=pm[:], lhsT=ones_bf[:],
                                         rhs=bias_sb["fc2b"][:],
                                         start=False, stop=True)
                    z2_t = z2_tiles[T * NSUB + s]
                    nc.vector.tensor_tensor(out=z2_t[:], in0=z2_t[:], in1=pm[:],
                                            op=OP.add)

        tc.no_sync_barrier()

        # ================= pass 3: LN3 + output conv =================
        with (
            tc.tile_pool(name="p3z3", bufs=6) as z3pool,
            tc.tile_pool(name="p3z3T", bufs=2) as z3Tpool,
            tc.tile_pool(name="p3o", bufs=4) as opool,
            tc.tile_pool(name="p3stats", bufs=4) as stats3,
            tc.tile_pool(name="p3tz", bufs=2, space="PSUM") as ps_tz3,
            tc.tile_pool(name="p3o_ps", bufs=2, space="PSUM") as ps_o,
        ):
            for T in range(NT):
                mv3 = stats3.tile([P, NSUB, 2], F32, tag="mv3")
                for s in range(NSUB):
                    st3 = stats3.tile([P, nc.vector.BN_STATS_DIM], F32, tag="st3")
                    nc.vector.bn_stats(out=st3[:], in_=z2_tiles[T * NSUB + s][:])
                    nc.vector.bn_aggr(out=mv3[:, s, :], in_=st3[:])
                rstd3 = _newton_rstd(nc, stats3, mv3[:, :, 1], NSUB)
                nmr3 = stats3.tile([P, NSUB], F32, tag="nmr3")
                nc.vector.tensor_tensor(out=nmr3[:], in0=mv3[:, :, 0],
                                        in1=rstd3[:], op=OP.mult)
                nc.vector.tensor_scalar(out=nmr3[:], in0=nmr3[:], scalar1=-1.0,
                                        scalar2=None, op0=OP.mult)
                z3_list = []
                for s in range(NSUB):
                    z3_t = z3pool.tile([P, D2], BF16, tag="z3")
                    nc.scalar.activation(out=z3_t[:],
                                         in_=z2_tiles[T * NSUB + s][:],
                                         func=AF.Identity,
                                         scale=rstd3[:, s:s + 1],
                                         bias=nmr3[:, s:s + 1])
                    z3_list.append(z3_t)
                z3T = z3Tpool.tile([P, DC, TW], BF16, tag="z3T")
                for f in range(DC):
                    tz = ps_tz3.tile([P, TW], BF16, tag="tz3")
                    for s in range(NSUB):
                        nc.tensor.transpose(
                            out=tz[:, s * P:(s + 1) * P],
                            in_=z3_list[s][:, f * P:(f + 1) * P],
                            identity=ident_b[:])
                    nc.any.tensor_copy(out=z3T[:, f, :], in_=tz[:])
                for cc in range(CC):
                    po = ps_o.tile([P, TW], F32, tag="o")
                    for f in range(DC):
                        nc.tensor.matmul(out=po[:],
                                         lhsT=conv_sb[:, f, cc * P:(cc + 1) * P],
                                         rhs=z3T[:, f, :],
                                         start=(f == 0), stop=(f == DC - 1))
                    ot = opool.tile([P, TW], F32, tag="ot")
                    if flags["ccb"]:
                        nc.any.tensor_scalar(out=ot[:], in0=po[:],
                                             scalar1=bias_sb["ccb"][:, cc:cc + 1],
                                             scalar2=None, op0=OP.add)
                    else:
                        nc.any.tensor_copy(out=ot[:], in_=po[:])
                    nc.sync.dma_start(
                        out=out_d[cc * P:(cc + 1) * P, T * TW:(T + 1) * TW],
                        in_=ot[:])

    nc.compile()
    return nc


_CACHE = {}


def _chunk(a, p=P):
    """[K, N] -> [P, K//P, N] (k-chunks on partitions)."""
    k, n = a.shape
    return np.ascontiguousarray(a.reshape(k // p, p, n).transpose(1, 0, 2))


def _prep_inputs_impl(x, cls, color_centers, semantic_centers, a_embed, b_embed,
                      ce_w, ce_b, sem_w, sem_b, q_w, q_b,
                      n1_w, n1_b, n2_w, n2_b, n3_w, n3_b,
                      fc1_w, fc1_b, fc2_w, fc2_b, conv_w, conv_b):
    f32 = lambda a: np.asarray(a, np.float32)
    bf = lambda a: np.ascontiguousarray(np.asarray(a, ml_dtypes.bfloat16))
    x = np.ascontiguousarray(f32(x))
    cls = f32(cls)
    color_centers = np.asarray(color_centers, np.int64)
    semantic_centers = f32(semantic_centers)
    a_embed, b_embed = f32(a_embed), f32(b_embed)
    ce_w, ce_b = f32(ce_w), f32(ce_b)
    sem_w, sem_b = f32(sem_w), f32(sem_b)
    q_w, q_b = f32(q_w), f32(q_b)
    n1_w, n1_b = f32(n1_w), f32(n1_b)
    n2_w, n2_b = f32(n2_w), f32(n2_b)
    n3_w, n3_b = f32(n3_w), f32(n3_b)
    fc1_w, fc1_b = f32(fc1_w), f32(fc1_b)
    fc2_w, fc2_b = f32(fc2_w), f32(fc2_b)
    conv_w, conv_b = f32(conv_w), f32(conv_b)

    # ---- host-side weight folding ----
    semP = semantic_centers @ sem_w + sem_b                  # [n, e]
    Wf = (n1_w[:, None] * q_w) @ semP.T                      # [c, n]
    ncsum = -Wf.sum(0)                                       # [n]
    crow = (n1_b @ q_w + q_b) @ semP.T                       # [n]
    ab = np.concatenate([a_embed[color_centers[:, :, 0]],
                         b_embed[color_centers[:, :, 1]]], -1)   # [4, n, 2ce]
    ce = np.einsum('inf,ifd->ind', ab, ce_w) + ce_b[:, None, :]  # [4, n, ce]
    colemb_all = np.einsum('ind,bi->bnd', ce, cls)               # [b, n, ce]
    conv_f = n3_w[:, None] * conv_w
    ccb = n3_b @ conv_w + conv_b

    nz = lambda a: bool(np.any(a != 0))
    flags = {
        "qcr": nz(crow),
        "c1": nz(fc1_b),
        "fc2b": nz(fc2_b),
        "ln2w": bool(np.any(n2_w != 1.0)),
        "ln2b": nz(n2_b),
        "ccb": nz(ccb),
    }

    wf_p = _chunk(Wf)                                        # [P, CC, NCOL] f32
    fc1_p = bf(_chunk(fc1_w))                                # [P, DC, D2]
    fc2_p = bf(_chunk(fc2_w))
    conv_p = bf(_chunk(conv_f))                              # [P, DC, C]
    ncsum_p = bf(ncsum[None, :])

    xn = x.reshape(B, C, S)
    in_maps = []
    for k in range(N_CORES):
        m = {
            "x": np.ascontiguousarray(xn[k]),
            "wf": wf_p,
            "ncsum": ncsum_p,
            "colemb": bf(_chunk(colemb_all[k])),             # [P, NC_, CE]
            "fc1": fc1_p,
            "fc2": fc2_p,
            "conv": conv_p,
        }
        if flags["qcr"]:
            m["crow"] = bf(crow[None, :])
        if flags["c1"]:
            m["c1b"] = np.ascontiguousarray(fc1_b.reshape(DC, P).T)
        if flags["fc2b"]:
            m["fc2b"] = bf(fc2_b[None, :])
        if flags["ln2w"]:
            m["ln2w"] = np.ascontiguousarray(np.broadcast_to(n2_w, (P, D2)))
        if flags["ln2b"]:
            m["ln2b"] = np.ascontiguousarray(np.broadcast_to(n2_b, (P, D2)))
        if flags["ccb"]:
            m["ccb"] = np.ascontiguousarray(ccb.reshape(CC, P).T)
        in_maps.append(m)
    return flags, in_maps


def run(flags, in_maps, **kw):
    key = tuple(sorted(flags.items()))
    if key not in _CACHE:
        _CACHE[key] = build_bass(flags)
    nc = _CACHE[key]
    res = run_bass_kernel_spmd(nc, in_maps, core_ids=list(range(N_CORES)), **kw)
    out = np.stack([res.results[k]["out"] for k in range(N_CORES)], axis=0)
    return out.reshape(B, C, H, W), res


def kernel(**inputs):
    flags, in_maps = _prep_inputs(**inputs)
    out, _ = run(flags, in_maps)
    return out


def _prep_inputs(x, cls, color_centers, semantic_centers, a_embed, b_embed,
                 ce_w, ce_b, sem_w, sem_b, q_w, q_b,
                 n1_w, n1_b, n2_w, n2_b, n3_w, n3_b,
                 fc1_w, fc1_b, fc2_w, fc2_b, conv_w, conv_b):
    return _prep_inputs_impl(
        x, cls, color_centers, semantic_centers, a_embed, b_embed,
        ce_w, ce_b, sem_w, sem_b, q_w, q_b,
        n1_w, n1_b, n2_w, n2_b, n3_w, n3_b,
        fc1_w, fc1_b, fc2_w, fc2_b, conv_w, conv_b)
